# Initial kernel scaffold
#
"""MoE runtime-experts kernel for 8 Trainium2 NeuronCores.

Expert-parallel: core e holds expert e's weights. Host routes tokens by
expert id (argsort), pads each expert batch to a common capacity C, and
each core computes y = gelu(x @ W1 + b1) @ W2 + b2 for its batch as
dense matmuls in a transposed layout:

    L1: hT[hid, tok]  = W1[in, hid].T-contract  xT[in, tok]
    L2: yT[out, tok]  = W2[hid, out].T-contract hT[hid, tok]

Weights stay stationary on the PE (lhsT), tokens are the moving dim, so
activations flow through both layers without any on-device transpose.
Inputs/weights are cast to bf16 on host (PSUM accumulates fp32).

All streamed tensors (x, w1, w2, biases) are pre-swizzled on the host to
partition-major layouts so every DMA window is ONE contiguous run per
SBUF partition (128 descriptors). Naive [IN, HID]-style layouts cost
32*128 descriptors per window, and the Sync engine's DMA_DIRECT2D issue
blocks on descriptor generation (~4ns/desc) — the profile showed 17us(!)
single issue instructions starving both the DMA queues and the PE.
"""

import numpy as np
import ml_dtypes

import concourse.bass as bass
import concourse.mybir as mybir
import concourse.tile as tile
from concourse import bacc
from concourse.bass_utils import run_bass_kernel_spmd

P = 128
N_CORES = 8
BF16 = mybir.dt.bfloat16
F32 = mybir.dt.float32

_nc_cache = {}


def _tile_widths(C):
    """Tile widths: (C//256 - 1) full 256-token tiles plus one overwide
    last tile of 256+(C%256), up to 511.

    The matmul moving dim is arbitrary, so rather than padding C and
    paying whole extra LDWEIGHTS-bound sweeps for the remainder
    (LDWEIGHTS cost is per-instruction, not per-token), the remainder
    tokens ride the last tile's existing instructions at ~0.5 cycle/row.
    256-wide tiles amortize the ~3-cycle per-matmul overhead over twice
    the rows vs 128 (measured ~56.1ns/128tok at W=128, ~53.1 at W=122 ->
    ~54.6/128tok at 256) and pack 2 PSUM chains per bank exactly."""
    n, e = divmod(C, 2 * P)
    if n == 0:
        return [C]
    if e == 0:
        return [2 * P] * n
    if e <= 16:
        # small remainder folds into one overwide tile; WMAX stays <=272
        # so the SBUF budget (h/x/o pools scale with WMAX) always fits
        return [2 * P] * (n - 1) + [2 * P + e]
    # larger remainder gets its own narrow tile (LDWEIGHTS-bound but
    # correct); an overwide 256+e tile would blow the SBUF budget
    return [2 * P] * n + [e]


def _ramp(total, first=1, cap=None):
    """Window sizes [1, 1, 2, 4, ...] (in m-tiles) doubling up to `cap`,
    summing to total. Small leading windows let dependent compute start
    early; big trailing windows amortize per-DMA issue overhead."""
    out, w, off = [], first, 0
    while off < total:
        take = min(w, total - off)
        out.append(take)
        off += take
        if len(out) >= 2:
            w *= 2
            if cap is not None:
                w = min(w, cap)
    return out


def _build_kernel(C, IN, HID, OUT, psum_bufs=8, repeat=1,
                  PIPE=2, GROUP=True, SPLIT_Y=True):
    K1, M1 = IN // P, HID // P
    K2, M2 = HID // P, OUT // P
    assert C % 16 == 0
    widths = _tile_widths(C)
    n_t = len(widths)
    starts_ = [sum(widths[:i]) for i in range(n_t)]
    xoffs = [K1 * s for s in starts_]
    # PSUM chains packed per 512-f32 bank: per-tile group size/stride.
    WMAX = max(widths)

    def _grp(W):
        gn = min(4, 512 // W)
        return gn, 512 // gn

    nc = bacc.Bacc("TRN2", target_bir_lowering=False, debug=False,
                   num_devices=N_CORES)
    # Swizzled layouts: tile t of xT is a [K1*W_t]-wide block per
    # partition, element (p, k*W_t+w) = x[token start_t+w, feature
    # k*128+p]; element (p, m, k*128+c) of w1 is w1[k*128+p, m*128+c];
    # likewise w2. Biases are [P, M] with (p, m) = b[m*128+p].
    xT = nc.dram_tensor("xT", [P, K1 * C], BF16, kind="ExternalInput")
    w1 = nc.dram_tensor("w1", [P, M1, K1 * P], BF16, kind="ExternalInput")
    w2 = nc.dram_tensor("w2", [P, M2, K2 * P], BF16, kind="ExternalInput")
    b1 = nc.dram_tensor("b1", [P, M1], F32, kind="ExternalInput")
    b2 = nc.dram_tensor("b2", [P, M2], F32, kind="ExternalInput")
    yT = nc.dram_tensor("yT", [OUT, C], F32, kind="ExternalOutput")

    with tile.TileContext(nc) as tc:
        with (
            tc.tile_pool(name="weights", bufs=1) as wpool,
            tc.tile_pool(name="xbuf",
                         bufs=(n_t if repeat == 1 else PIPE + 1)) as xpool,
            tc.tile_pool(name="hbuf", bufs=PIPE + 1) as hpool,
            tc.tile_pool(name="obuf",
                         bufs=(1 if WMAX > 200 else 2)) as opool,
            tc.tile_pool(name="psum", bufs=psum_bufs, space="PSUM") as pspool,
        ):
            w1_sb = wpool.tile([P, M1, K1 * P], BF16)
            w2_sb = wpool.tile([P, M2, K2 * P], BF16)
            b1_sb = wpool.tile([P, M1], F32)
            b2_sb = wpool.tile([P, M2], F32)

            depth = min(PIPE, n_t)
            # Single-shot: preload ALL x tiles into dedicated buffers (x is
            # only 2.3MB total). The first `depth` tiles go in front of the
            # weights so the L1 prefix can start immediately; the rest are
            # queued behind the weights (needed much later). DMA queue
            # order is issue order, so this sequencing is what keeps the
            # in-order Tensor stream fed.
            def _dma_x(x_sb, it, eng=None):
                nw = K1 * widths[it]
                (eng or nc.sync).dma_start(
                    x_sb[:, :nw], xT.ap()[:, xoffs[it]:xoffs[it] + nw])

            # Everything rides the single sync HWDGE queue in need order:
            # the first chain needs only x0 + w1's first m-window, the
            # depth-PIPE interleaved prefix consumes w1 m-columns at
            # ~560/PIPE GB/s (below the ~270GB/s the queue delivers), and
            # the first L2 chain (which needs w2's head) only runs after
            # the prefix. Cross-queue splits (scalar HWDGE) were tried and
            # lose to fair-share contention against this critical stream.
            # Every window is one contiguous run per partition on both
            # sides (128 descriptors).
            w1_windows = _ramp(M1, cap=4)
            x_tiles = {}
            if repeat == 1:
                for it in range(n_t):
                    x_tiles[it] = xpool.tile([P, K1 * WMAX], BF16,
                                             tag="x", name="x_sb")
                # x0 and w1's first m-window are split in two so the very
                # first chain's k=0 matmul (which range-tracks only the
                # leading halves) starts after ~320KB. The trailing x
                # pieces ride the SCALAR HWDGE queue: they're small enough
                # not to disturb the sync queue's w1 stream, and the two
                # queues deliver the 1MB startup x in parallel (the trace
                # showed both early PE gaps waiting on x, not w1).
                hw = K1 * widths[0] // 2
                nc.sync.dma_start(x_tiles[0][:, :hw], xT.ap()[:, :hw])
            w10 = w1_windows[0]
            hm = K1 * P // 2
            nc.sync.dma_start(w1_sb[:, :w10, :hm], w1.ap()[:, :w10, :hm])
            if repeat == 1:
                nc.scalar.dma_start(x_tiles[0][:, hw:K1 * widths[0]],
                                    xT.ap()[:, hw:K1 * widths[0]])
            nc.sync.dma_start(w1_sb[:, :w10, hm:], w1.ap()[:, :w10, hm:])
            nc.sync.dma_start(b1_sb[:], b1.ap())
            if repeat == 1:
                for it in range(1, depth):
                    _dma_x(x_tiles[it], it, eng=nc.scalar)
            nc.sync.dma_start(b2_sb[:], b2.ap())
            off = w1_windows[0]
            for w in w1_windows[1:]:
                nc.sync.dma_start(w1_sb[:, off:off + w],
                                  w1.ap()[:, off:off + w])
                off += w
            if repeat == 1:
                for it in range(depth, n_t):
                    _dma_x(x_tiles[it], it)
            off = 0
            for w in _ramp(M2, cap=2):
                nc.sync.dma_start(w2_sb[:, off:off + w],
                                  w2.ap()[:, off:off + w])
                off += w

            def l1_phase(it):
                x_sb = _get_x(it)
                W = widths[it]
                h_sb = hpool.tile([P, M1, WMAX], BF16, tag="h",
                                  name="h_sb")
                GN, SW = _grp(W)
                # Pack GN accumulation chains into one PSUM bank so the
                # slot-WAR sem wait is paid once per GN chains.
                for mg in range(0, M1, GN):
                    grp = range(mg, min(mg + GN, M1))
                    ps = pspool.tile([P, 512], F32, tag="ps", name="ps")
                    for mi, m in enumerate(grp):
                        for k in range(K1):
                            nc.tensor.matmul(
                                ps[:, mi * SW:mi * SW + W],
                                w1_sb[:, m, bass.ts(k, P)],
                                x_sb[:, k * W:(k + 1) * W],
                                start=(k == 0),
                                stop=(k == K1 - 1),
                            )
                    for mi, m in enumerate(grp):
                        nc.scalar.activation(
                            h_sb[:, m, :W],
                            ps[:, mi * SW:mi * SW + W],
                            mybir.ActivationFunctionType.Gelu,
                            bias=b1_sb[:, m:m + 1],
                        )
                return h_sb

            yTr = yT.ap().rearrange("(m p) c -> p m c", p=P)

            def l2_phase(it, h_sb, is_last=False):
                n0, W = starts_[it], widths[it]
                o_sb = opool.tile([P, M2, WMAX], F32, tag="o",
                                  name="o_sb")
                GN, SW = _grp(W)
                for mg in range(0, M2, GN):
                    grp = range(mg, min(mg + GN, M2))
                    ps = pspool.tile([P, 512], F32, tag="ps", name="ps")
                    for mi, m in enumerate(grp):
                        for k in range(K2):
                            nc.tensor.matmul(
                                ps[:, mi * SW:mi * SW + W],
                                w2_sb[:, m, bass.ts(k, P)],
                                h_sb[:, k, :W],
                                start=(k == 0),
                                stop=(k == K2 - 1),
                            )
                    for mi, m in enumerate(grp):
                        nc.vector.tensor_tensor(
                            o_sb[:, m, :W],
                            ps[:, mi * SW:mi * SW + W],
                            b2_sb[:, m:m + 1].to_broadcast((P, W)),
                            mybir.AluOpType.add,
                        )
                if SPLIT_Y and is_last:
                    # Tail trim: per-m out-DMAs so the final DMA covers only
                    # the last m-tile instead of the whole o_sb.
                    for m in range(M2):
                        nc.sync.dma_start(yTr[:, m, n0:n0 + W],
                                          o_sb[:, m, :W])
                else:
                    nc.sync.dma_start(yTr[:, :, n0:n0 + W], o_sb[:, :, :W])

            def _get_x(it):
                if it in x_tiles:
                    return x_tiles[it]
                x_sb = xpool.tile([P, K1 * WMAX], BF16, tag="x",
                                  name="x_sb")
                _dma_x(x_sb, it)
                return x_sb

            def l1_prefix(depth, hs):
                # m-interleaved L1 over the first `depth` tiles: one tile's
                # chains consume w1 m-columns at ~560GB/s, faster than the
                # ~430GB/s the DMA delivers w1 at startup. Spreading each
                # m-window over `depth` chains keeps the PE behind the DMA
                # so the weight load is fully hidden.
                xs = {it: _get_x(it) for it in range(depth)}
                for it in range(depth):
                    hs[it] = hpool.tile([P, M1, WMAX], BF16, tag="h",
                                        name="h_sb")
                GN, SW = _grp(max(widths[it] for it in range(depth)))
                chains = [(m, it) for m in range(M1) for it in range(depth)]
                for g in range(0, len(chains), GN):
                    grp = chains[g:g + GN]
                    ps = pspool.tile([P, 512], F32, tag="ps", name="ps")
                    for mi, (m, it) in enumerate(grp):
                        W = widths[it]
                        for k in range(K1):
                            nc.tensor.matmul(
                                ps[:, mi * SW:mi * SW + W],
                                w1_sb[:, m, bass.ts(k, P)],
                                xs[it][:, k * W:(k + 1) * W],
                                start=(k == 0),
                                stop=(k == K1 - 1),
                            )
                    for mi, (m, it) in enumerate(grp):
                        nc.scalar.activation(
                            hs[it][:, m, :widths[it]],
                            ps[:, mi * SW:mi * SW + widths[it]],
                            mybir.ActivationFunctionType.Gelu,
                            bias=b1_sb[:, m:m + 1],
                        )

            def body():
                # Software pipeline: L1 runs PIPE tiles ahead of L2 so the
                # w2 weight DMA tail hides behind L1 compute at startup.
                hs = {}
                l1_prefix(depth, hs)
                for j in range(n_t):
                    if j + depth < n_t:
                        hs[j + depth] = l1_phase(j + depth)
                    l2_phase(j, hs.pop(j), is_last=(j == n_t - 1))

            if repeat == 1:
                body()
            else:
                with tc.For_i(0, repeat, 1, name="rep"):
                    body()
    nc.compile()
    return nc


def _get_kernel(C, IN, HID, OUT):
    key = (C, IN, HID, OUT)
    if key not in _nc_cache:
        _nc_cache[key] = _build_kernel(C, IN, HID, OUT)
    return _nc_cache[key]


def prepare_in_maps(inputs):
    """Host-side routing: sort tokens by expert, pad to capacity C,
    build per-core swizzled input maps. Returns (in_maps, meta)."""
    x = np.ascontiguousarray(np.asarray(inputs["x"], dtype=np.float32))
    idx = np.asarray(inputs["indices_s"]).astype(np.int64)
    w1 = np.asarray(inputs["weight1"], dtype=np.float32)
    w2 = np.asarray(inputs["weight2"], dtype=np.float32)
    b1 = np.asarray(inputs["bias1"], dtype=np.float32)
    b2 = np.asarray(inputs["bias2"], dtype=np.float32)

    T = x.shape[0]
    E, IN, HID = w1.shape
    OUT = w2.shape[2]
    K1, M1 = IN // P, HID // P
    K2, M2 = HID // P, OUT // P
    assert E == N_CORES
    bf = ml_dtypes.bfloat16

    order = np.argsort(idx, kind="stable")
    counts = np.bincount(idx, minlength=E)
    starts = np.zeros(E + 1, dtype=np.int64)
    starts[1:] = np.cumsum(counts)
    # Capacity rounds to 16 (not 128); tiles split C near-equally.
    C = max(-(-int(counts.max()) // 16) * 16, 16)
    widths = _tile_widths(C)

    xbf = x.astype(bf)
    in_maps = []
    for e in range(E):
        toks = order[starts[e]:starts[e + 1]]
        xp = np.zeros((C, IN), dtype=bf)
        if len(toks):
            xp[:len(toks)] = xbf[toks]
        # per-tile blocks: (n0+w, k*128+p) -> [p, k*W+w], concatenated
        blocks = []
        n0 = 0
        for W in widths:
            blk = xp[n0:n0 + W].reshape(W, K1, P).transpose(2, 1, 0)
            blocks.append(blk.reshape(P, K1 * W))
            n0 += W
        x_sw = np.ascontiguousarray(np.concatenate(blocks, axis=1))
        # (k*128+p, m*128+c) -> [p, m, k*128+c]
        w1_sw = np.ascontiguousarray(
            w1[e].astype(bf).reshape(K1, P, M1, P).transpose(1, 2, 0, 3)
        ).reshape(P, M1, K1 * P)
        w2_sw = np.ascontiguousarray(
            w2[e].astype(bf).reshape(K2, P, M2, P).transpose(1, 2, 0, 3)
        ).reshape(P, M2, K2 * P)
        in_maps.append({
            "xT": x_sw,
            "w1": w1_sw,
            "w2": w2_sw,
            # biases pre-transposed to [P, M]
            "b1": np.ascontiguousarray(b1[e].reshape(M1, P).T),
            "b2": np.ascontiguousarray(b2[e].reshape(M2, P).T),
        })
    meta = {"key": (C, IN, HID, OUT), "order": order, "starts": starts,
            "T": T, "OUT": OUT}
    return in_maps, meta


def scatter_output(inputs, yT_all, meta):
    """Scatter per-core yT [E, OUT, C] back to [T, 1, OUT] fp32."""
    order, starts = meta["order"], meta["starts"]
    out = np.empty((meta["T"], meta["OUT"]), dtype=np.float32)
    for e in range(N_CORES):
        toks = order[starts[e]:starts[e + 1]]
        if len(toks):
            out[toks] = yT_all[e][:, :len(toks)].T
    return out[:, None, :]


def kernel(**inputs):
    in_maps, meta = prepare_in_maps(inputs)
    nc = _get_kernel(*meta["key"])
    res = run_bass_kernel_spmd(nc, in_maps, core_ids=list(range(N_CORES)),
                               trace=False)
    yT_all = np.stack([res.results[e]["yT"] for e in range(N_CORES)])
    return scatter_output(inputs, yT_all, meta)



# revision 1
# speedup vs baseline: 1.2015x; 1.2015x over previous
"""MoE runtime-experts kernel for 8 Trainium2 NeuronCores.

Expert-parallel: core e holds expert e's weights. Host routes tokens by
expert id (argsort), pads each expert batch to a common capacity C, and
each core computes y = gelu(x @ W1 + b1) @ W2 + b2 for its batch as
dense matmuls in a transposed layout:

    L1: hT[hid, tok]  = W1[in, hid].T-contract  xT[in, tok]
    L2: yT[out, tok]  = W2[hid, out].T-contract hT[hid, tok]

Weights stay stationary on the PE (lhsT), tokens are the moving dim, so
activations flow through both layers without any on-device transpose.
Inputs/weights are cast to bf16 on host (PSUM accumulates fp32).

All streamed tensors (x, w1, w2, biases) are pre-swizzled on the host to
partition-major layouts so every DMA window is ONE contiguous run per
SBUF partition (128 descriptors). Naive [IN, HID]-style layouts cost
32*128 descriptors per window, and the Sync engine's DMA_DIRECT2D issue
blocks on descriptor generation (~4ns/desc) — the profile showed 17us(!)
single issue instructions starving both the DMA queues and the PE.
"""

import numpy as np
import ml_dtypes

import concourse.bass as bass
import concourse.mybir as mybir
import concourse.tile as tile
from concourse import bacc
from concourse.bass_utils import run_bass_kernel_spmd

P = 128
N_CORES = 8
BF16 = mybir.dt.bfloat16
F32 = mybir.dt.float32

_nc_cache = {}


def _tile_widths(C):
    """Tile widths: (C//256 - 1) full 256-token tiles plus one overwide
    last tile of 256+(C%256), up to 511.

    The matmul moving dim is arbitrary, so rather than padding C and
    paying whole extra LDWEIGHTS-bound sweeps for the remainder
    (LDWEIGHTS cost is per-instruction, not per-token), the remainder
    tokens ride the last tile's existing instructions at ~0.5 cycle/row.
    256-wide tiles amortize the ~3-cycle per-matmul overhead over twice
    the rows vs 128 (measured ~56.1ns/128tok at W=128, ~53.1 at W=122 ->
    ~54.6/128tok at 256) and pack 2 PSUM chains per bank exactly."""
    n, e = divmod(C, 2 * P)
    if n == 0:
        return [C]
    if e == 0:
        return [2 * P] * n
    if e <= 16:
        # small remainder folds into one overwide tile; WMAX stays <=272
        # so the SBUF budget (h/x/o pools scale with WMAX) always fits
        return [2 * P] * (n - 1) + [2 * P + e]
    # larger remainder gets its own narrow tile (LDWEIGHTS-bound but
    # correct); an overwide 256+e tile would blow the SBUF budget
    return [2 * P] * n + [e]


def _ramp(total, first=1, cap=None):
    """Window sizes [1, 1, 2, 4, ...] (in m-tiles) doubling up to `cap`,
    summing to total. Small leading windows let dependent compute start
    early; big trailing windows amortize per-DMA issue overhead."""
    out, w, off = [], first, 0
    while off < total:
        take = min(w, total - off)
        out.append(take)
        off += take
        if len(out) >= 2:
            w *= 2
            if cap is not None:
                w = min(w, cap)
    return out


def _build_kernel(C, IN, HID, OUT, psum_bufs=8, repeat=1,
                  PIPE=2, GROUP=True, SPLIT_Y=True):
    K1, M1 = IN // P, HID // P
    K2, M2 = HID // P, OUT // P
    assert C % 16 == 0
    widths = _tile_widths(C)
    n_t = len(widths)
    starts_ = [sum(widths[:i]) for i in range(n_t)]
    xoffs = [K1 * s for s in starts_]
    # PSUM chains packed per 512-f32 bank: per-tile group size/stride.
    WMAX = max(widths)

    def _grp(W):
        gn = min(4, 512 // W)
        return gn, 512 // gn

    nc = bacc.Bacc("TRN2", target_bir_lowering=False, debug=False,
                   num_devices=N_CORES)
    # Swizzled layouts: tile t of xT is a [K1*W_t]-wide block per
    # partition, element (p, k*W_t+w) = x[token start_t+w, feature
    # k*128+p]; element (p, m, k*128+c) of w1 is w1[k*128+p, m*128+c];
    # likewise w2. Biases are [P, M] with (p, m) = b[m*128+p].
    xT = nc.dram_tensor("xT", [P, K1 * C], BF16, kind="ExternalInput")
    w1 = nc.dram_tensor("w1", [P, M1, K1 * P], BF16, kind="ExternalInput")
    w2 = nc.dram_tensor("w2", [P, M2, K2 * P], BF16, kind="ExternalInput")
    b1 = nc.dram_tensor("b1", [P, M1], F32, kind="ExternalInput")
    b2 = nc.dram_tensor("b2", [P, M2], F32, kind="ExternalInput")
    yT = nc.dram_tensor("yT", [OUT, C], F32, kind="ExternalOutput")

    with tile.TileContext(nc) as tc:
        with (
            tc.tile_pool(name="weights", bufs=1) as wpool,
            tc.tile_pool(name="xbuf",
                         bufs=(n_t if repeat == 1 else PIPE + 1)) as xpool,
            tc.tile_pool(name="hbuf", bufs=PIPE + 1) as hpool,
            tc.tile_pool(name="obuf",
                         bufs=(1 if WMAX > 200 else 2)) as opool,
            tc.tile_pool(name="psum", bufs=psum_bufs, space="PSUM") as pspool,
        ):
            w1_sb = wpool.tile([P, M1, K1 * P], BF16)
            w2_sb = wpool.tile([P, M2, K2 * P], BF16)
            b1_sb = wpool.tile([P, M1], F32)
            b2_sb = wpool.tile([P, M2], F32)

            depth = min(PIPE, n_t)
            # Single-shot: preload ALL x tiles into dedicated buffers (x is
            # only 2.3MB total). The first `depth` tiles go in front of the
            # weights so the L1 prefix can start immediately; the rest are
            # queued behind the weights (needed much later). DMA queue
            # order is issue order, so this sequencing is what keeps the
            # in-order Tensor stream fed.
            def _dma_x(x_sb, it, eng=None):
                nw = K1 * widths[it]
                (eng or nc.sync).dma_start(
                    x_sb[:, :nw], xT.ap()[:, xoffs[it]:xoffs[it] + nw])

            # Everything rides the single sync HWDGE queue in need order:
            # the first chain needs only x0 + w1's first m-window, the
            # depth-PIPE interleaved prefix consumes w1 m-columns at
            # ~560/PIPE GB/s (below the ~270GB/s the queue delivers), and
            # the first L2 chain (which needs w2's head) only runs after
            # the prefix. Cross-queue splits (scalar HWDGE) were tried and
            # lose to fair-share contention against this critical stream.
            # Every window is one contiguous run per partition on both
            # sides (128 descriptors).
            w1_windows = _ramp(M1, cap=4)
            x_tiles = {}
            if repeat == 1:
                for it in range(n_t):
                    x_tiles[it] = xpool.tile([P, K1 * WMAX], BF16,
                                             tag="x", name="x_sb")
                # x0 and w1's first m-window are split in two so the very
                # first chain's k=0 matmul (which range-tracks only the
                # leading halves) starts after ~320KB. The trailing x
                # pieces ride the SCALAR HWDGE queue: they're small enough
                # not to disturb the sync queue's w1 stream, and the two
                # queues deliver the 1MB startup x in parallel (the trace
                # showed both early PE gaps waiting on x, not w1).
                hw = K1 * widths[0] // 2
                nc.sync.dma_start(x_tiles[0][:, :hw], xT.ap()[:, :hw])
            w10 = w1_windows[0]
            hm = K1 * P // 2
            nc.sync.dma_start(w1_sb[:, :w10, :hm], w1.ap()[:, :w10, :hm])
            if repeat == 1:
                nc.scalar.dma_start(x_tiles[0][:, hw:K1 * widths[0]],
                                    xT.ap()[:, hw:K1 * widths[0]])
            nc.sync.dma_start(w1_sb[:, :w10, hm:], w1.ap()[:, :w10, hm:])
            nc.sync.dma_start(b1_sb[:], b1.ap())
            if repeat == 1:
                for it in range(1, depth):
                    _dma_x(x_tiles[it], it, eng=nc.scalar)
            nc.sync.dma_start(b2_sb[:], b2.ap())
            off = w1_windows[0]
            for w in w1_windows[1:]:
                nc.sync.dma_start(w1_sb[:, off:off + w],
                                  w1.ap()[:, off:off + w])
                off += w
            if repeat == 1:
                for it in range(depth, n_t):
                    _dma_x(x_tiles[it], it)
            off = 0
            for w in _ramp(M2, cap=2):
                nc.sync.dma_start(w2_sb[:, off:off + w],
                                  w2.ap()[:, off:off + w])
                off += w

            def l1_phase(it):
                x_sb = _get_x(it)
                W = widths[it]
                h_sb = hpool.tile([P, M1, WMAX], BF16, tag="h",
                                  name="h_sb")
                GN, SW = _grp(W)
                # Pack GN accumulation chains into one PSUM bank so the
                # slot-WAR sem wait is paid once per GN chains.
                for mg in range(0, M1, GN):
                    grp = range(mg, min(mg + GN, M1))
                    ps = pspool.tile([P, 512], F32, tag="ps", name="ps")
                    for mi, m in enumerate(grp):
                        for k in range(K1):
                            nc.tensor.matmul(
                                ps[:, mi * SW:mi * SW + W],
                                w1_sb[:, m, bass.ts(k, P)],
                                x_sb[:, k * W:(k + 1) * W],
                                start=(k == 0),
                                stop=(k == K1 - 1),
                            )
                    for mi, m in enumerate(grp):
                        nc.scalar.activation(
                            h_sb[:, m, :W],
                            ps[:, mi * SW:mi * SW + W],
                            mybir.ActivationFunctionType.Gelu,
                            bias=b1_sb[:, m:m + 1],
                        )
                return h_sb

            yTr = yT.ap().rearrange("(m p) c -> p m c", p=P)

            def l2_phase(it, h_sb, is_last=False):
                n0, W = starts_[it], widths[it]
                o_sb = opool.tile([P, M2, WMAX], F32, tag="o",
                                  name="o_sb")
                GN, SW = _grp(W)
                for mg in range(0, M2, GN):
                    grp = range(mg, min(mg + GN, M2))
                    ps = pspool.tile([P, 512], F32, tag="ps", name="ps")
                    for mi, m in enumerate(grp):
                        for k in range(K2):
                            nc.tensor.matmul(
                                ps[:, mi * SW:mi * SW + W],
                                w2_sb[:, m, bass.ts(k, P)],
                                h_sb[:, k, :W],
                                start=(k == 0),
                                stop=(k == K2 - 1),
                            )
                    for mi, m in enumerate(grp):
                        nc.vector.tensor_tensor(
                            o_sb[:, m, :W],
                            ps[:, mi * SW:mi * SW + W],
                            b2_sb[:, m:m + 1].to_broadcast((P, W)),
                            mybir.AluOpType.add,
                        )
                if SPLIT_Y and is_last:
                    # Tail trim: per-m out-DMAs so the final DMA covers only
                    # the last m-tile instead of the whole o_sb.
                    for m in range(M2):
                        nc.sync.dma_start(yTr[:, m, n0:n0 + W],
                                          o_sb[:, m, :W])
                else:
                    nc.sync.dma_start(yTr[:, :, n0:n0 + W], o_sb[:, :, :W])

            def _get_x(it):
                if it in x_tiles:
                    return x_tiles[it]
                x_sb = xpool.tile([P, K1 * WMAX], BF16, tag="x",
                                  name="x_sb")
                _dma_x(x_sb, it)
                return x_sb

            def l1_prefix(depth, hs):
                # m-interleaved L1 over the first `depth` tiles: one tile's
                # chains consume w1 m-columns at ~560GB/s, faster than the
                # ~430GB/s the DMA delivers w1 at startup. Spreading each
                # m-window over `depth` chains keeps the PE behind the DMA
                # so the weight load is fully hidden.
                xs = {it: _get_x(it) for it in range(depth)}
                for it in range(depth):
                    hs[it] = hpool.tile([P, M1, WMAX], BF16, tag="h",
                                        name="h_sb")
                GN, SW = _grp(max(widths[it] for it in range(depth)))
                chains = [(m, it) for m in range(M1) for it in range(depth)]
                for g in range(0, len(chains), GN):
                    grp = chains[g:g + GN]
                    ps = pspool.tile([P, 512], F32, tag="ps", name="ps")
                    for mi, (m, it) in enumerate(grp):
                        W = widths[it]
                        for k in range(K1):
                            nc.tensor.matmul(
                                ps[:, mi * SW:mi * SW + W],
                                w1_sb[:, m, bass.ts(k, P)],
                                xs[it][:, k * W:(k + 1) * W],
                                start=(k == 0),
                                stop=(k == K1 - 1),
                            )
                    for mi, (m, it) in enumerate(grp):
                        nc.scalar.activation(
                            hs[it][:, m, :widths[it]],
                            ps[:, mi * SW:mi * SW + widths[it]],
                            mybir.ActivationFunctionType.Gelu,
                            bias=b1_sb[:, m:m + 1],
                        )

            def body():
                # Software pipeline: L1 runs PIPE tiles ahead of L2 so the
                # w2 weight DMA tail hides behind L1 compute at startup.
                hs = {}
                l1_prefix(depth, hs)
                for j in range(n_t):
                    if j + depth < n_t:
                        hs[j + depth] = l1_phase(j + depth)
                    l2_phase(j, hs.pop(j), is_last=(j == n_t - 1))

            if repeat == 1:
                body()
            else:
                with tc.For_i(0, repeat, 1, name="rep"):
                    body()
    nc.compile()
    return nc


def _get_kernel(C, IN, HID, OUT):
    key = (C, IN, HID, OUT)
    if key not in _nc_cache:
        _nc_cache[key] = _build_kernel(C, IN, HID, OUT)
    return _nc_cache[key]


def prepare_in_maps(inputs):
    """Host-side routing: sort tokens by expert, pad to capacity C,
    build per-core swizzled input maps. Returns (in_maps, meta)."""
    x = np.ascontiguousarray(np.asarray(inputs["x"], dtype=np.float32))
    idx = np.asarray(inputs["indices_s"]).astype(np.int64)
    w1 = np.asarray(inputs["weight1"], dtype=np.float32)
    w2 = np.asarray(inputs["weight2"], dtype=np.float32)
    b1 = np.asarray(inputs["bias1"], dtype=np.float32)
    b2 = np.asarray(inputs["bias2"], dtype=np.float32)

    T = x.shape[0]
    E, IN, HID = w1.shape
    OUT = w2.shape[2]
    K1, M1 = IN // P, HID // P
    K2, M2 = HID // P, OUT // P
    assert E == N_CORES
    bf = ml_dtypes.bfloat16

    order = np.argsort(idx, kind="stable")
    counts = np.bincount(idx, minlength=E)
    starts = np.zeros(E + 1, dtype=np.int64)
    starts[1:] = np.cumsum(counts)
    # Capacity rounds to 16 (not 128); tiles split C near-equally.
    C = max(-(-int(counts.max()) // 16) * 16, 16)
    widths = _tile_widths(C)

    xbf = x.astype(bf)
    in_maps = []
    for e in range(E):
        toks = order[starts[e]:starts[e + 1]]
        xp = np.zeros((C, IN), dtype=bf)
        if len(toks):
            xp[:len(toks)] = xbf[toks]
        # per-tile blocks: (n0+w, k*128+p) -> [p, k*W+w], concatenated
        blocks = []
        n0 = 0
        for W in widths:
            blk = xp[n0:n0 + W].reshape(W, K1, P).transpose(2, 1, 0)
            blocks.append(blk.reshape(P, K1 * W))
            n0 += W
        x_sw = np.ascontiguousarray(np.concatenate(blocks, axis=1))
        # (k*128+p, m*128+c) -> [p, m, k*128+c]
        w1_sw = np.ascontiguousarray(
            w1[e].astype(bf).reshape(K1, P, M1, P).transpose(1, 2, 0, 3)
        ).reshape(P, M1, K1 * P)
        w2_sw = np.ascontiguousarray(
            w2[e].astype(bf).reshape(K2, P, M2, P).transpose(1, 2, 0, 3)
        ).reshape(P, M2, K2 * P)
        in_maps.append({
            "xT": x_sw,
            "w1": w1_sw,
            "w2": w2_sw,
            # biases pre-transposed to [P, M]
            "b1": np.ascontiguousarray(b1[e].reshape(M1, P).T),
            "b2": np.ascontiguousarray(b2[e].reshape(M2, P).T),
        })
    meta = {"key": (C, IN, HID, OUT), "order": order, "starts": starts,
            "T": T, "OUT": OUT}
    return in_maps, meta


def scatter_output(inputs, yT_all, meta):
    """Scatter per-core yT [E, OUT, C] back to [T, 1, OUT] fp32."""
    order, starts = meta["order"], meta["starts"]
    out = np.empty((meta["T"], meta["OUT"]), dtype=np.float32)
    for e in range(N_CORES):
        toks = order[starts[e]:starts[e + 1]]
        if len(toks):
            out[toks] = yT_all[e][:, :len(toks)].T
    return out[:, None, :]


def kernel(**inputs):
    in_maps, meta = prepare_in_maps(inputs)
    nc = _get_kernel(*meta["key"])
    res = run_bass_kernel_spmd(nc, in_maps, core_ids=list(range(N_CORES)),
                               trace=False)
    yT_all = np.stack([res.results[e]["yT"] for e in range(N_CORES)])
    return scatter_output(inputs, yT_all, meta)

